# revision 15
# baseline (speedup 1.0000x reference)
"""Trainium2 Bass kernel for conv-augmented causal multi-head attention.

Module: q/k/v = Linear(x) (k scaled), depthwise conv(K=3) along L, causal MHA,
output projection. Returns (out [B,L,D], attn [B,H,L,L]).

Sharding: tensor-parallel over the 16 heads -> 2 heads per core (8 cores).
Each core computes its q/k/v slices [128, B*L] (transposed layout), runs
attention for its two heads with sim held TRANSPOSED ([k, q]) so that
  - attn @ v is a plain matmul with v in natural [k, d] layout (built via a
    PE transpose), and
  - the softmax denominator falls out of the same matmul via a ones column.
The wo partial products are summed on the host; attn slices are returned
transposed and swapped back on the host.

All matmuls run as float32r (full fp32 storage, relaxed-precision PE mode,
4x faster than strict fp32).
"""

import numpy as np

import concourse.bass as bass
import concourse.bacc as bacc_mod
import concourse.mybir as mybir
from concourse.tile import TileContext
from concourse.bass_utils import run_bass_kernel_spmd

B, L, D, H = 2, 2048, 1024, 16
HD = D // H                     # 64
NCORES = 8
HPC = H // NCORES               # 2 heads per core
DL = HPC * HD                   # 128 local channels per core
SCALE = HD ** -0.5
BL = B * L                      # 4096
F32 = mybir.dt.float32
F32R = mybir.dt.float32r

LT = 512                        # projection l-tile (moving dim)
NLT = BL // LT                  # 8
QC = 512                        # attention q chunk
NQC = L // QC                   # 4 per batch
KT = 128                        # attention k tile (partition dim)
NKT = L // KT                   # 16 per batch
NDC = D // 128                  # 8 contraction chunks


def _r(ap):
    return ap.bitcast(F32R)


def build_program():
    nc = bacc_mod.Bacc(None, target_bir_lowering=False, debug=False)

    xT = nc.declare_dram_parameter("xT", [D, BL], F32R, isOutput=False)
    wqT = nc.declare_dram_parameter("wqT", [D, DL], F32R, isOutput=False)
    wkT = nc.declare_dram_parameter("wkT", [D, DL], F32R, isOutput=False)
    wvT = nc.declare_dram_parameter("wvT", [D, DL], F32R, isOutput=False)
    cw = nc.declare_dram_parameter("cw", [3, DL, 3], F32, isOutput=False)
    cb = nc.declare_dram_parameter("cb", [3, DL, 1], F32, isOutput=False)
    woT = nc.declare_dram_parameter("woT", [HPC, HD, D], F32R, isOutput=False)
    masks = nc.declare_dram_parameter("masks", [4, 128, QC], F32R, isOutput=False)
    ident128 = nc.declare_dram_parameter("ident128", [128, 128], F32R, isOutput=False)
    ident = nc.declare_dram_parameter("ident", [128, HD], F32R, isOutput=False)
    ones_r = nc.declare_dram_parameter("ones_r", [1, 128], F32, isOutput=False)
    ones_col = nc.declare_dram_parameter("ones_col", [128, 1], F32R, isOutput=False)

    attnT = nc.declare_dram_parameter("attnT", [B, HPC, L, L], F32, isOutput=True)
    outT = nc.declare_dram_parameter("outT", [D, BL], F32, isOutput=True)

    with TileContext(nc) as tc:
        with tc.tile_pool(name="consts", bufs=1) as consts:
            # constants
            wsb = []
            for i, w in enumerate((wqT, wkT, wvT)):
                t = consts.tile([128, NDC, DL], F32R, name=f"w_{i}")
                nc.sync.dma_start(out=t, in_=w.rearrange("(dc p) e -> p dc e", p=128))
                wsb.append(t)
            cw_sb = consts.tile([DL, 3, 3], F32)
            nc.sync.dma_start(out=cw_sb, in_=cw.rearrange("m p k -> p m k"))
            cb_sb = consts.tile([DL, 3, 1], F32)
            nc.sync.dma_start(out=cb_sb, in_=cb.rearrange("m p k -> p m k"))
            wo_sb = []
            for h in range(HPC):
                t = consts.tile([HD, D], F32R, name=f"wo_{h}")
                nc.sync.dma_start(out=t, in_=woT[h])
                wo_sb.append(t)
            masks_sb = consts.tile([128, 4, QC], F32R)
            nc.sync.dma_start(out=masks_sb, in_=masks.rearrange("m p q -> p m q"))
            ident_sb = consts.tile([128, HD], F32R)
            nc.sync.dma_start(out=ident_sb, in_=ident[:, :])
            id128_sb = consts.tile([128, 128], F32R)
            nc.sync.dma_start(out=id128_sb, in_=ident128[:, :])
            ones_sb = consts.tile([1, 128], F32)
            nc.sync.dma_start(out=ones_sb, in_=ones_r[:, :])
            zero_sb = consts.tile([128, 1024], F32)

            # persistent projection outputs [128ch, B*L]
            qkvT = []
            for i in range(3):
                t = consts.tile([128, BL], F32R, name=f"qkvT_{i}")
                qkvT.append(t)

            # ---------------- projections ----------------
            with tc.tile_pool(name="proj_x", bufs=2) as xpool, \
                 tc.tile_pool(name="proj_ps", bufs=2, space="PSUM") as projps, \
                 tc.tile_pool(name="conv_tmp", bufs=2) as ctmp, \
                 tc.tile_pool(name="conv_tmp2", bufs=2) as ctmp2:

                def conv_one(pi, b):
                    w0 = cw_sb[:, pi, 0:1]
                    w1 = cw_sb[:, pi, 1:2]
                    w2 = cw_sb[:, pi, 2:3]
                    bias = cb_sb[:, pi, :]
                    xb = qkvT[pi][:, b * L:(b + 1) * L]
                    tmp = ctmp.tile([128, L], F32, tag="tmp")
                    tmp2 = ctmp2.tile([128, L - 1], F32, tag="tmp2")
                    # center tap + bias
                    nc.vector.tensor_scalar(
                        out=tmp, in0=xb, scalar1=w1, scalar2=bias,
                        op0=mybir.AluOpType.mult, op1=mybir.AluOpType.add)
                    # right tap: y[l] += w2 * x[l+1]  (l in 0..L-2)
                    nc.vector.tensor_scalar(
                        out=tmp2, in0=xb[:, 1:L], scalar1=w2, scalar2=None,
                        op0=mybir.AluOpType.mult)
                    nc.vector.tensor_add(tmp[:, 0:L - 1], tmp[:, 0:L - 1], tmp2)
                    # left tap: y[l] += w0 * x[l-1]  (l in 1..L-1)
                    nc.vector.tensor_scalar(
                        out=tmp2, in0=xb[:, 0:L - 1], scalar1=w0, scalar2=None,
                        op0=mybir.AluOpType.mult)
                    nc.vector.tensor_add(tmp[:, 1:L], tmp[:, 1:L], tmp2)
                    nc.scalar.copy(xb, tmp)

                for lt in range(NLT):
                    xt = xpool.tile([128, NDC, LT], F32R)
                    nc.sync.dma_start(
                        out=xt,
                        in_=xT.rearrange("(dc p) l -> p dc l", p=128)[
                            :, :, lt * LT:(lt + 1) * LT],
                    )
                    for pi in range(3):
                        ps = projps.tile([128, LT], F32)
                        for dc in range(NDC):
                            nc.tensor.matmul(
                                ps, wsb[pi][:, dc, :], xt[:, dc, :],
                                start=(dc == 0), stop=(dc == NDC - 1),
                            )
                        nc.scalar.copy(qkvT[pi][:, lt * LT:(lt + 1) * LT], ps)
                    if lt == 1:
                        # gate the attn zero-fill DMAs behind early projection
                        # output so they don't steal HBM bandwidth from the
                        # xT loads that feed the PE
                        nc.vector.tensor_scalar(
                            out=zero_sb, in0=qkvT[0][:, 0:1024],
                            scalar1=0.0, scalar2=None,
                            op0=mybir.AluOpType.mult)
                    if lt == NLT // 2 - 1:
                        # batch 0 fully projected: conv it now (v first — the
                        # attention phase consumes v earliest via transposes)
                        for pi in (2, 0, 1):
                            conv_one(pi, 0)
                for pi in (2, 0, 1):
                    conv_one(pi, 1)

            qT, kT, vT = qkvT

            # ---------------- attention ----------------
            with tc.tile_pool(name="v1p", bufs=2) as v1p, \
                 tc.tile_pool(name="expp", bufs=2) as expp, \
                 tc.tile_pool(name="sps", bufs=2, space="PSUM") as simps, \
                 tc.tile_pool(name="cps", bufs=2, space="PSUM") as ctxps, \
                 tc.tile_pool(name="ops", bufs=2, space="PSUM") as opps, \
                 tc.tile_pool(name="recp", bufs=2) as recp, \
                 tc.tile_pool(name="ctxtp", bufs=2) as ctxtp, \
                 tc.tile_pool(name="osbp", bufs=2) as osbp:
                for b in range(B):
                    # zero-fill of fully-masked attn regions (k > q), this batch
                    for h in range(HPC):
                        att_b = attnT[b, h].rearrange("(kt p) q -> p kt q", p=128)
                        for kt in range(4, NKT):
                            w = QC * (kt // 4)  # fully-masked columns 0..w
                            c0 = 0
                            while c0 < w:
                                c1 = min(c0 + 1024, w)
                                nc.gpsimd.dma_start(
                                    out=att_b[:, kt, c0:c1],
                                    in_=zero_sb[:, 0:c1 - c0])
                                c0 = c1

                    # natural-layout v tiles (+ ones column) for both heads
                    v1 = v1p.tile([128, HPC, NKT, HD + 1], F32R)
                    oc = ones_col[:, :]
                    oc_b = bass.AP(tensor=oc.tensor, offset=oc.offset,
                                   ap=[oc.ap[0], [0, HPC * NKT]])
                    v1_col = bass.AP(tensor=v1.tensor, offset=v1.offset + HD,
                                     ap=[v1.ap[0], [HD + 1, HPC * NKT]])
                    nc.sync.dma_start(out=v1_col, in_=oc_b)
                    for h in range(HPC):
                        for kt in range(NKT):
                            tp = simps.tile([128, HD], F32R, tag="sps")
                            nc.tensor.transpose(
                                tp,
                                vT[h * HD:(h + 1) * HD,
                                   b * L + kt * KT: b * L + (kt + 1) * KT],
                                ident_sb[h * HD:(h + 1) * HD, :])
                            nc.vector.tensor_copy(v1[:, h, kt, 0:HD], tp)

                    for a in range(NQC):
                        nkt = 4 * a + 4      # causal: k-tiles 0..nkt-1 computed
                        qsl = slice(b * L + a * QC, b * L + (a + 1) * QC)
                        ctx_t = ctxtp.tile([HD, HPC, QC], F32R)
                        for h in range(HPC):
                            hsl = slice(h * HD, (h + 1) * HD)
                            ebuf = expp.tile([128, nkt, QC], F32R, tag="ebuf")
                            for kp in range(0, nkt, 2):
                                sps = simps.tile([128, 2, QC], F32, tag="sps")
                                for j in range(2):
                                    kt = kp + j
                                    ksl = slice(b * L + kt * KT,
                                                b * L + (kt + 1) * KT)
                                    diag = kt >= nkt - 4
                                    nc.tensor.matmul(
                                        sps[:, j, :],
                                        kT[hsl, ksl], qT[hsl, qsl],
                                        start=True, stop=not diag)
                                    if diag:
                                        # sim += -1e30 on masked entries
                                        nc.tensor.matmul(
                                            sps[:, j, :], id128_sb,
                                            masks_sb[:, kt - (nkt - 4), :],
                                            start=False, stop=True)
                                nc.scalar.activation(
                                    ebuf[:, kp:kp + 2, :], sps,
                                    mybir.ActivationFunctionType.Exp)
                            # ctxt^T (rows 0..63) + softmax denominator (row 64)
                            cps = ctxps.tile([128, QC], F32)
                            for kt in range(nkt):
                                nc.tensor.matmul(
                                    cps[0:HD + 1, :],
                                    v1[:, h, kt, :], ebuf[:, kt, :],
                                    start=(kt == 0), stop=(kt == nkt - 1))
                            # NB: reciprocal_approx_fast mis-reads nonzero base
                            # partitions on HW — copy the sum row to base 0 first.
                            srow = recp.tile([1, QC], F32, tag="srow")
                            nc.vector.tensor_copy(srow, cps[HD:HD + 1, :])
                            rec = recp.tile([1, QC], F32, tag="rec")
                            nc.vector.reciprocal_approx_fast(rec, srow)
                            # broadcast 1/sum across partitions via PE outer product
                            bc = opps.tile([128, QC], F32, tag="ops")
                            nc.tensor.matmul(bc, ones_sb, rec,
                                             start=True, stop=True)
                            # stage 1/sum broadcast in SBUF (gpsimd cannot
                            # read PSUM); normalize + write out per 4-k-tile
                            # chunk, alternating DVE/gpsimd, so ebuf drains
                            # (and its pool slot frees) progressively
                            bcs = recp.tile([128, QC], F32, tag="bcs")
                            nc.vector.tensor_copy(bcs, bc)
                            att_d = attnT[b, h].rearrange(
                                "(kt p) q -> p kt q", p=128)
                            for ci, c0 in enumerate(range(0, nkt, 4)):
                                bcs_b = bass.AP(
                                    tensor=bcs.tensor, offset=bcs.offset,
                                    ap=[bcs.ap[0], [0, 4], bcs.ap[1]])
                                nc.vector.tensor_mul(
                                    ebuf[:, c0:c0 + 4, :],
                                    ebuf[:, c0:c0 + 4, :], bcs_b)
                                nc.sync.dma_start(
                                    out=att_d[:, c0:c0 + 4,
                                              a * QC:(a + 1) * QC],
                                    in_=ebuf[:, c0:c0 + 4, :].bitcast(F32))
                            # normalized ctxt^T for this head / q-chunk
                            nc.vector.tensor_mul(
                                ctx_t[:, h, :], cps[0:HD, :], bcs[0:HD, :])
                        # wo partial for this (b, q-chunk): contract local d
                        osb = None
                        for ec in range(NDC):
                            ops = opps.tile([128, QC], F32, tag="ops")
                            for h in range(HPC):
                                nc.tensor.matmul(
                                    ops,
                                    wo_sb[h][:, ec * 128:(ec + 1) * 128],
                                    ctx_t[:, h, :],
                                    start=(h == 0), stop=(h == HPC - 1))
                            if ec % 4 == 0:
                                osb = osbp.tile([128, 4, QC], F32, tag="osb")
                            nc.scalar.copy(osb[:, ec % 4, :], ops)
                            if ec % 4 == 3:
                                nc.sync.dma_start(
                                    out=outT.rearrange(
                                        "(ec p) l -> p ec l", p=128)[
                                            :, ec - 3:ec + 1,
                                            b * L + a * QC: b * L + (a + 1) * QC],
                                    in_=osb)
    nc.compile()
    return nc


_NC_CACHE = None


def _get_program():
    global _NC_CACHE
    if _NC_CACHE is None:
        _NC_CACHE = build_program()
    return _NC_CACHE


def make_in_maps(x, wq, wk, wv, wo, cq_w, cq_b, ck_w, ck_b, cv_w, cv_b):
    x2 = np.ascontiguousarray(x, dtype=np.float32).reshape(BL, D)
    xT = np.ascontiguousarray(x2.T)
    masks = np.zeros((4, 128, QC), dtype=np.float32)
    for d in range(4):
        p = np.arange(128)[:, None]
        j = np.arange(QC)[None, :]
        masks[d] = np.where(j >= p + 128 * d, 0.0, -1e30).astype(np.float32)
    ident = np.ascontiguousarray(np.tile(np.eye(HD, dtype=np.float32), (2, 1)))
    ones_r = np.ones((1, 128), dtype=np.float32)
    ones_col = np.ones((128, 1), dtype=np.float32)

    wk_s = (np.asarray(wk, dtype=np.float32) * SCALE)
    in_maps = []
    for c in range(NCORES):
        sl = slice(c * DL, (c + 1) * DL)
        woT = np.ascontiguousarray(
            np.stack([np.asarray(wo, dtype=np.float32)[:, c * DL + h * HD:
                                                       c * DL + (h + 1) * HD].T
                      for h in range(HPC)]))
        in_maps.append({
            "xT": xT,
            "wqT": np.ascontiguousarray(np.asarray(wq, dtype=np.float32)[sl].T),
            "wkT": np.ascontiguousarray(wk_s[sl].T),
            "wvT": np.ascontiguousarray(np.asarray(wv, dtype=np.float32)[sl].T),
            "cw": np.ascontiguousarray(np.stack([
                np.asarray(cq_w, dtype=np.float32)[sl, 0, :],
                np.asarray(ck_w, dtype=np.float32)[sl, 0, :],
                np.asarray(cv_w, dtype=np.float32)[sl, 0, :]])),
            "cb": np.ascontiguousarray(np.stack([
                np.asarray(cq_b, dtype=np.float32)[sl, None],
                np.asarray(ck_b, dtype=np.float32)[sl, None],
                np.asarray(cv_b, dtype=np.float32)[sl, None]])),
            "woT": woT,
            "masks": masks,
            "ident": ident,
            "ident128": np.eye(128, dtype=np.float32),
            "ones_r": ones_r,
            "ones_col": ones_col,
        })
    return in_maps


def gather(results):
    out_acc = np.zeros((D, BL), dtype=np.float64)
    attn = np.empty((B, H, L, L), dtype=np.float32)
    for c, res in enumerate(results):
        out_acc += res["outT"]
        attn[:, HPC * c:HPC * (c + 1)] = res["attnT"].transpose(0, 1, 3, 2)
    out = out_acc.astype(np.float32).T.reshape(B, L, D)
    return out, attn


def run(inputs, **spmd_kwargs):
    nc = _get_program()
    in_maps = make_in_maps(**inputs)
    br = run_bass_kernel_spmd(nc, in_maps, list(range(NCORES)), **spmd_kwargs)
    out, attn = gather(br.results)
    return (out, attn), br


def kernel(**inputs):
    (out, attn), _ = run(inputs)
    return out, attn


# revision 16
# speedup vs baseline: 1.0085x; 1.0085x over previous
"""Trainium2 Bass kernel for conv-augmented causal multi-head attention.

Module: q/k/v = Linear(x) (k scaled), depthwise conv(K=3) along L, causal MHA,
output projection. Returns (out [B,L,D], attn [B,H,L,L]).

Sharding: tensor-parallel over the 16 heads -> 2 heads per core (8 cores).
Each core computes its q/k/v slices [128, B*L] (transposed layout), runs
attention for its two heads with sim held TRANSPOSED ([k, q]) so that
  - attn @ v is a plain matmul with v in natural [k, d] layout (built via a
    PE transpose), and
  - the softmax denominator falls out of the same matmul via a ones column.
The wo partial products are summed on the host; attn slices are returned
transposed and swapped back on the host.

All matmuls run as float32r (full fp32 storage, relaxed-precision PE mode,
4x faster than strict fp32).
"""

import numpy as np

import concourse.bass as bass
import concourse.bacc as bacc_mod
import concourse.mybir as mybir
from concourse.tile import TileContext
from concourse.bass_utils import run_bass_kernel_spmd

B, L, D, H = 2, 2048, 1024, 16
HD = D // H                     # 64
NCORES = 8
HPC = H // NCORES               # 2 heads per core
DL = HPC * HD                   # 128 local channels per core
SCALE = HD ** -0.5
BL = B * L                      # 4096
F32 = mybir.dt.float32
F32R = mybir.dt.float32r

LT = 512                        # projection l-tile (moving dim)
NLT = BL // LT                  # 8
QC = 512                        # attention q chunk
NQC = L // QC                   # 4 per batch
KT = 128                        # attention k tile (partition dim)
NKT = L // KT                   # 16 per batch
NDC = D // 128                  # 8 contraction chunks


def _r(ap):
    return ap.bitcast(F32R)


def build_program():
    nc = bacc_mod.Bacc(None, target_bir_lowering=False, debug=False)

    xT = nc.declare_dram_parameter("xT", [D, BL], F32R, isOutput=False)
    wqT = nc.declare_dram_parameter("wqT", [D, DL], F32R, isOutput=False)
    wkT = nc.declare_dram_parameter("wkT", [D, DL], F32R, isOutput=False)
    wvT = nc.declare_dram_parameter("wvT", [D, DL], F32R, isOutput=False)
    cw = nc.declare_dram_parameter("cw", [3, DL, 3], F32, isOutput=False)
    cb = nc.declare_dram_parameter("cb", [3, DL, 1], F32, isOutput=False)
    woT = nc.declare_dram_parameter("woT", [HPC, HD, D], F32R, isOutput=False)
    masks = nc.declare_dram_parameter("masks", [4, 128, QC], F32R, isOutput=False)
    ident128 = nc.declare_dram_parameter("ident128", [128, 128], F32R, isOutput=False)
    ident = nc.declare_dram_parameter("ident", [128, HD], F32R, isOutput=False)
    ones_r = nc.declare_dram_parameter("ones_r", [1, 128], F32, isOutput=False)
    ones_col = nc.declare_dram_parameter("ones_col", [128, 1], F32R, isOutput=False)

    attnT = nc.declare_dram_parameter("attnT", [B, HPC, L, L], F32, isOutput=True)
    outT = nc.declare_dram_parameter("outT", [D, BL], F32, isOutput=True)

    with TileContext(nc) as tc:
        with tc.tile_pool(name="consts", bufs=1) as consts:
            # constants
            wsb = []
            for i, w in enumerate((wqT, wkT, wvT)):
                t = consts.tile([128, NDC, DL], F32R, name=f"w_{i}")
                nc.sync.dma_start(out=t, in_=w.rearrange("(dc p) e -> p dc e", p=128))
                wsb.append(t)
            cw_sb = consts.tile([DL, 3, 3], F32)
            nc.sync.dma_start(out=cw_sb, in_=cw.rearrange("m p k -> p m k"))
            cb_sb = consts.tile([DL, 3, 1], F32)
            nc.sync.dma_start(out=cb_sb, in_=cb.rearrange("m p k -> p m k"))
            wo_sb = []
            for h in range(HPC):
                t = consts.tile([HD, D], F32R, name=f"wo_{h}")
                nc.sync.dma_start(out=t, in_=woT[h])
                wo_sb.append(t)
            masks_sb = consts.tile([128, 4, QC], F32R)
            nc.sync.dma_start(out=masks_sb, in_=masks.rearrange("m p q -> p m q"))
            ident_sb = consts.tile([128, HD], F32R)
            nc.sync.dma_start(out=ident_sb, in_=ident[:, :])
            id128_sb = consts.tile([128, 128], F32R)
            nc.sync.dma_start(out=id128_sb, in_=ident128[:, :])
            ones_sb = consts.tile([1, 128], F32)
            nc.sync.dma_start(out=ones_sb, in_=ones_r[:, :])
            zero_sb = consts.tile([128, 1024], F32)

            # persistent projection outputs [128ch, B*L]
            qkvT = []
            for i in range(3):
                t = consts.tile([128, BL], F32R, name=f"qkvT_{i}")
                qkvT.append(t)

            # ---------------- projections ----------------
            with tc.tile_pool(name="proj_x", bufs=2) as xpool, \
                 tc.tile_pool(name="proj_ps", bufs=2, space="PSUM") as projps, \
                 tc.tile_pool(name="conv_tmp", bufs=2) as ctmp, \
                 tc.tile_pool(name="conv_tmp2", bufs=2) as ctmp2:

                def conv_one(pi, b):
                    w0 = cw_sb[:, pi, 0:1]
                    w1 = cw_sb[:, pi, 1:2]
                    w2 = cw_sb[:, pi, 2:3]
                    bias = cb_sb[:, pi, :]
                    xb = qkvT[pi][:, b * L:(b + 1) * L]
                    tmp = ctmp.tile([128, L], F32, tag="tmp")
                    tmp2 = ctmp2.tile([128, L - 1], F32, tag="tmp2")
                    # center tap + bias
                    nc.vector.tensor_scalar(
                        out=tmp, in0=xb, scalar1=w1, scalar2=bias,
                        op0=mybir.AluOpType.mult, op1=mybir.AluOpType.add)
                    # right tap: y[l] += w2 * x[l+1]  (l in 0..L-2)
                    nc.vector.tensor_scalar(
                        out=tmp2, in0=xb[:, 1:L], scalar1=w2, scalar2=None,
                        op0=mybir.AluOpType.mult)
                    nc.vector.tensor_add(tmp[:, 0:L - 1], tmp[:, 0:L - 1], tmp2)
                    # left tap: y[l] += w0 * x[l-1]  (l in 1..L-1)
                    nc.vector.tensor_scalar(
                        out=tmp2, in0=xb[:, 0:L - 1], scalar1=w0, scalar2=None,
                        op0=mybir.AluOpType.mult)
                    nc.vector.tensor_add(tmp[:, 1:L], tmp[:, 1:L], tmp2)
                    nc.scalar.copy(xb, tmp)

                for lt in range(NLT):
                    xt = xpool.tile([128, NDC, LT], F32R)
                    nc.sync.dma_start(
                        out=xt,
                        in_=xT.rearrange("(dc p) l -> p dc l", p=128)[
                            :, :, lt * LT:(lt + 1) * LT],
                    )
                    for pi in range(3):
                        ps = projps.tile([128, LT], F32)
                        for dc in range(NDC):
                            nc.tensor.matmul(
                                ps, wsb[pi][:, dc, :], xt[:, dc, :],
                                start=(dc == 0), stop=(dc == NDC - 1),
                            )
                        nc.scalar.copy(qkvT[pi][:, lt * LT:(lt + 1) * LT], ps)
                    if lt == 1:
                        # gate the attn zero-fill DMAs behind early projection
                        # output so they don't steal HBM bandwidth from the
                        # xT loads that feed the PE
                        nc.vector.tensor_scalar(
                            out=zero_sb, in0=qkvT[0][:, 0:1024],
                            scalar1=0.0, scalar2=None,
                            op0=mybir.AluOpType.mult)
                    if lt == NLT // 2 - 1:
                        # batch 0 fully projected: conv it now (v first — the
                        # attention phase consumes v earliest via transposes)
                        for pi in (2, 0, 1):
                            conv_one(pi, 0)
                for pi in (2, 0, 1):
                    conv_one(pi, 1)

            qT, kT, vT = qkvT

            # ---------------- attention ----------------
            with tc.tile_pool(name="v1p", bufs=2) as v1p, \
                 tc.tile_pool(name="expp", bufs=2) as expp, \
                 tc.tile_pool(name="sps", bufs=2, space="PSUM") as simps, \
                 tc.tile_pool(name="cps", bufs=2, space="PSUM") as ctxps, \
                 tc.tile_pool(name="ops", bufs=2, space="PSUM") as opps, \
                 tc.tile_pool(name="recp", bufs=2) as recp, \
                 tc.tile_pool(name="ctxtp", bufs=2) as ctxtp, \
                 tc.tile_pool(name="osbp", bufs=2) as osbp:
                for b in range(B):
                    # zero-fill of fully-masked attn regions (k > q), this batch
                    for h in range(HPC):
                        att_b = attnT[b, h].rearrange("(kt p) q -> p kt q", p=128)
                        for kt in range(4, NKT):
                            w = QC * (kt // 4)  # fully-masked columns 0..w
                            c0 = 0
                            while c0 < w:
                                c1 = min(c0 + 1024, w)
                                nc.gpsimd.dma_start(
                                    out=att_b[:, kt, c0:c1],
                                    in_=zero_sb[:, 0:c1 - c0])
                                c0 = c1

                    # natural-layout v tiles (+ ones column) for both heads
                    v1 = v1p.tile([128, HPC, NKT, HD + 1], F32R)
                    oc = ones_col[:, :]
                    oc_b = bass.AP(tensor=oc.tensor, offset=oc.offset,
                                   ap=[oc.ap[0], [0, HPC * NKT]])
                    v1_col = bass.AP(tensor=v1.tensor, offset=v1.offset + HD,
                                     ap=[v1.ap[0], [HD + 1, HPC * NKT]])
                    nc.sync.dma_start(out=v1_col, in_=oc_b)
                    for h in range(HPC):
                        for kt in range(NKT):
                            tp = simps.tile([128, HD], F32R, tag="sps")
                            nc.tensor.transpose(
                                tp,
                                vT[h * HD:(h + 1) * HD,
                                   b * L + kt * KT: b * L + (kt + 1) * KT],
                                ident_sb[h * HD:(h + 1) * HD, :])
                            nc.vector.tensor_copy(v1[:, h, kt, 0:HD], tp)

                    for a in range(NQC):
                        nkt = 4 * a + 4      # causal: k-tiles 0..nkt-1 computed
                        qsl = slice(b * L + a * QC, b * L + (a + 1) * QC)
                        ctx_t = ctxtp.tile([HD, HPC, QC], F32R)
                        # stage 1: sim + exp for BOTH heads (ACT-rate limited),
                        # then stage 2: both ctxt chains back-to-back — a dense
                        # PE block that keeps the HAM clock warm and overlaps
                        # head-0's recip/normalize tail with head-1's matmuls
                        ebufs = []
                        for h in range(HPC):
                            hsl = slice(h * HD, (h + 1) * HD)
                            ebuf = expp.tile([128, nkt, QC], F32R, tag="ebuf")
                            ebufs.append(ebuf)
                            for kp in range(0, nkt, 2):
                                sps = simps.tile([128, 2, QC], F32, tag="sps")
                                for j in range(2):
                                    kt = kp + j
                                    ksl = slice(b * L + kt * KT,
                                                b * L + (kt + 1) * KT)
                                    diag = kt >= nkt - 4
                                    nc.tensor.matmul(
                                        sps[:, j, :],
                                        kT[hsl, ksl], qT[hsl, qsl],
                                        start=True, stop=not diag)
                                    if diag:
                                        # sim += -1e30 on masked entries
                                        nc.tensor.matmul(
                                            sps[:, j, :], id128_sb,
                                            masks_sb[:, kt - (nkt - 4), :],
                                            start=False, stop=True)
                                nc.scalar.activation(
                                    ebuf[:, kp:kp + 2, :], sps,
                                    mybir.ActivationFunctionType.Exp)
                        for h in range(HPC):
                            ebuf = ebufs[h]
                            # ctxt^T (rows 0..63) + softmax denominator (row 64)
                            cps = ctxps.tile([128, QC], F32)
                            for kt in range(nkt):
                                nc.tensor.matmul(
                                    cps[0:HD + 1, :],
                                    v1[:, h, kt, :], ebuf[:, kt, :],
                                    start=(kt == 0), stop=(kt == nkt - 1))
                            # NB: reciprocal_approx_fast mis-reads nonzero base
                            # partitions on HW — copy the sum row to base 0 first.
                            srow = recp.tile([1, QC], F32, tag="srow")
                            nc.vector.tensor_copy(srow, cps[HD:HD + 1, :])
                            rec = recp.tile([1, QC], F32, tag="rec")
                            nc.vector.reciprocal_approx_fast(rec, srow)
                            # broadcast 1/sum across partitions via PE outer product
                            bc = opps.tile([128, QC], F32, tag="ops")
                            nc.tensor.matmul(bc, ones_sb, rec,
                                             start=True, stop=True)
                            bcs = recp.tile([128, QC], F32, tag="bcs")
                            nc.vector.tensor_copy(bcs, bc)
                            att_d = attnT[b, h].rearrange(
                                "(kt p) q -> p kt q", p=128)
                            for c0 in range(0, nkt, 4):
                                bcs_b = bass.AP(
                                    tensor=bcs.tensor, offset=bcs.offset,
                                    ap=[bcs.ap[0], [0, 4], bcs.ap[1]])
                                nc.vector.tensor_mul(
                                    ebuf[:, c0:c0 + 4, :],
                                    ebuf[:, c0:c0 + 4, :], bcs_b)
                                nc.sync.dma_start(
                                    out=att_d[:, c0:c0 + 4,
                                              a * QC:(a + 1) * QC],
                                    in_=ebuf[:, c0:c0 + 4, :].bitcast(F32))
                            # normalized ctxt^T for this head / q-chunk
                            nc.vector.tensor_mul(
                                ctx_t[:, h, :], cps[0:HD, :], bcs[0:HD, :])
                        # wo partial for this (b, q-chunk): contract local d
                        osb = None
                        for ec in range(NDC):
                            ops = opps.tile([128, QC], F32, tag="ops")
                            for h in range(HPC):
                                nc.tensor.matmul(
                                    ops,
                                    wo_sb[h][:, ec * 128:(ec + 1) * 128],
                                    ctx_t[:, h, :],
                                    start=(h == 0), stop=(h == HPC - 1))
                            if ec % 4 == 0:
                                osb = osbp.tile([128, 4, QC], F32, tag="osb")
                            nc.scalar.copy(osb[:, ec % 4, :], ops)
                            if ec % 4 == 3:
                                nc.sync.dma_start(
                                    out=outT.rearrange(
                                        "(ec p) l -> p ec l", p=128)[
                                            :, ec - 3:ec + 1,
                                            b * L + a * QC: b * L + (a + 1) * QC],
                                    in_=osb)
    nc.compile()
    return nc


_NC_CACHE = None


def _get_program():
    global _NC_CACHE
    if _NC_CACHE is None:
        _NC_CACHE = build_program()
    return _NC_CACHE


def make_in_maps(x, wq, wk, wv, wo, cq_w, cq_b, ck_w, ck_b, cv_w, cv_b):
    x2 = np.ascontiguousarray(x, dtype=np.float32).reshape(BL, D)
    xT = np.ascontiguousarray(x2.T)
    masks = np.zeros((4, 128, QC), dtype=np.float32)
    for d in range(4):
        p = np.arange(128)[:, None]
        j = np.arange(QC)[None, :]
        masks[d] = np.where(j >= p + 128 * d, 0.0, -1e30).astype(np.float32)
    ident = np.ascontiguousarray(np.tile(np.eye(HD, dtype=np.float32), (2, 1)))
    ones_r = np.ones((1, 128), dtype=np.float32)
    ones_col = np.ones((128, 1), dtype=np.float32)

    wk_s = (np.asarray(wk, dtype=np.float32) * SCALE)
    in_maps = []
    for c in range(NCORES):
        sl = slice(c * DL, (c + 1) * DL)
        woT = np.ascontiguousarray(
            np.stack([np.asarray(wo, dtype=np.float32)[:, c * DL + h * HD:
                                                       c * DL + (h + 1) * HD].T
                      for h in range(HPC)]))
        in_maps.append({
            "xT": xT,
            "wqT": np.ascontiguousarray(np.asarray(wq, dtype=np.float32)[sl].T),
            "wkT": np.ascontiguousarray(wk_s[sl].T),
            "wvT": np.ascontiguousarray(np.asarray(wv, dtype=np.float32)[sl].T),
            "cw": np.ascontiguousarray(np.stack([
                np.asarray(cq_w, dtype=np.float32)[sl, 0, :],
                np.asarray(ck_w, dtype=np.float32)[sl, 0, :],
                np.asarray(cv_w, dtype=np.float32)[sl, 0, :]])),
            "cb": np.ascontiguousarray(np.stack([
                np.asarray(cq_b, dtype=np.float32)[sl, None],
                np.asarray(ck_b, dtype=np.float32)[sl, None],
                np.asarray(cv_b, dtype=np.float32)[sl, None]])),
            "woT": woT,
            "masks": masks,
            "ident": ident,
            "ident128": np.eye(128, dtype=np.float32),
            "ones_r": ones_r,
            "ones_col": ones_col,
        })
    return in_maps


def gather(results):
    out_acc = np.zeros((D, BL), dtype=np.float64)
    attn = np.empty((B, H, L, L), dtype=np.float32)
    for c, res in enumerate(results):
        out_acc += res["outT"]
        attn[:, HPC * c:HPC * (c + 1)] = res["attnT"].transpose(0, 1, 3, 2)
    out = out_acc.astype(np.float32).T.reshape(B, L, D)
    return out, attn


def run(inputs, **spmd_kwargs):
    nc = _get_program()
    in_maps = make_in_maps(**inputs)
    br = run_bass_kernel_spmd(nc, in_maps, list(range(NCORES)), **spmd_kwargs)
    out, attn = gather(br.results)
    return (out, attn), br


def kernel(**inputs):
    (out, attn), _ = run(inputs)
    return out, attn


# revision 17
# speedup vs baseline: 1.0356x; 1.0269x over previous
"""Trainium2 Bass kernel for conv-augmented causal multi-head attention.

Module: q/k/v = Linear(x) (k scaled), depthwise conv(K=3) along L, causal MHA,
output projection. Returns (out [B,L,D], attn [B,H,L,L]).

Sharding: tensor-parallel over the 16 heads -> 2 heads per core (8 cores).
Each core computes its q/k/v slices [128, B*L] (transposed layout), runs
attention for its two heads with sim held TRANSPOSED ([k, q]) so that
  - attn @ v is a plain matmul with v in natural [k, d] layout (built via a
    PE transpose), and
  - the softmax denominator falls out of the same matmul via a ones column.
The wo partial products are summed on the host; attn slices are returned
transposed and swapped back on the host.

All matmuls run as float32r (full fp32 storage, relaxed-precision PE mode,
4x faster than strict fp32).
"""

import numpy as np

import concourse.bass as bass
import concourse.bacc as bacc_mod
import concourse.mybir as mybir
from concourse.tile import TileContext
from concourse.bass_utils import run_bass_kernel_spmd

B, L, D, H = 2, 2048, 1024, 16
HD = D // H                     # 64
NCORES = 8
HPC = H // NCORES               # 2 heads per core
DL = HPC * HD                   # 128 local channels per core
SCALE = HD ** -0.5
BL = B * L                      # 4096
F32 = mybir.dt.float32
F32R = mybir.dt.float32r

LT = 512                        # projection l-tile (moving dim)
NLT = BL // LT                  # 8
QC = 512                        # attention q chunk
NQC = L // QC                   # 4 per batch
KT = 128                        # attention k tile (partition dim)
NKT = L // KT                   # 16 per batch
NDC = D // 128                  # 8 contraction chunks


def _r(ap):
    return ap.bitcast(F32R)


def build_program():
    nc = bacc_mod.Bacc(None, target_bir_lowering=False, debug=False)

    xT = nc.declare_dram_parameter("xT", [D, BL], F32R, isOutput=False)
    wqT = nc.declare_dram_parameter("wqT", [D, DL], F32R, isOutput=False)
    wkT = nc.declare_dram_parameter("wkT", [D, DL], F32R, isOutput=False)
    wvT = nc.declare_dram_parameter("wvT", [D, DL], F32R, isOutput=False)
    cw = nc.declare_dram_parameter("cw", [3, DL, 3], F32, isOutput=False)
    cb = nc.declare_dram_parameter("cb", [3, DL, 1], F32, isOutput=False)
    woT = nc.declare_dram_parameter("woT", [HPC, HD, D], F32R, isOutput=False)
    masks = nc.declare_dram_parameter("masks", [4, 128, QC], F32R, isOutput=False)
    ident128 = nc.declare_dram_parameter("ident128", [128, 128], F32R, isOutput=False)
    ident = nc.declare_dram_parameter("ident", [128, HD], F32R, isOutput=False)
    ones_r = nc.declare_dram_parameter("ones_r", [1, 128], F32, isOutput=False)
    ones_col = nc.declare_dram_parameter("ones_col", [128, 1], F32R, isOutput=False)

    attnT = nc.declare_dram_parameter("attnT", [B, HPC, L, L], F32, isOutput=True)
    outT = nc.declare_dram_parameter("outT", [D, BL], F32, isOutput=True)

    with TileContext(nc) as tc:
        with tc.tile_pool(name="consts", bufs=1) as consts:
            # constants
            wsb = []
            for i, w in enumerate((wqT, wkT, wvT)):
                t = consts.tile([128, NDC, DL], F32R, name=f"w_{i}")
                nc.sync.dma_start(out=t, in_=w.rearrange("(dc p) e -> p dc e", p=128))
                wsb.append(t)
            cw_sb = consts.tile([DL, 3, 3], F32)
            nc.sync.dma_start(out=cw_sb, in_=cw.rearrange("m p k -> p m k"))
            cb_sb = consts.tile([DL, 3, 1], F32)
            nc.sync.dma_start(out=cb_sb, in_=cb.rearrange("m p k -> p m k"))
            wo_sb = []
            for h in range(HPC):
                t = consts.tile([HD, D], F32R, name=f"wo_{h}")
                nc.sync.dma_start(out=t, in_=woT[h])
                wo_sb.append(t)
            masks_sb = consts.tile([128, 4, QC], F32R)
            nc.sync.dma_start(out=masks_sb, in_=masks.rearrange("m p q -> p m q"))
            ident_sb = consts.tile([128, HD], F32R)
            nc.sync.dma_start(out=ident_sb, in_=ident[:, :])
            id128_sb = consts.tile([128, 128], F32R)
            nc.sync.dma_start(out=id128_sb, in_=ident128[:, :])
            ones_sb = consts.tile([1, 128], F32)
            nc.sync.dma_start(out=ones_sb, in_=ones_r[:, :])
            zero_sb = consts.tile([128, 1024], F32)

            # persistent projection outputs [128ch, B*L]
            qkvT = []
            for i in range(3):
                t = consts.tile([128, BL], F32R, name=f"qkvT_{i}")
                qkvT.append(t)

            # ---------------- projections ----------------
            with tc.tile_pool(name="proj_x", bufs=2) as xpool, \
                 tc.tile_pool(name="proj_ps", bufs=2, space="PSUM") as projps, \
                 tc.tile_pool(name="conv_tmp", bufs=2) as ctmp, \
                 tc.tile_pool(name="conv_tmp2", bufs=2) as ctmp2:

                def conv_one(pi, b):
                    w0 = cw_sb[:, pi, 0:1]
                    w1 = cw_sb[:, pi, 1:2]
                    w2 = cw_sb[:, pi, 2:3]
                    bias = cb_sb[:, pi, :]
                    xb = qkvT[pi][:, b * L:(b + 1) * L]
                    tmp = ctmp.tile([128, L], F32, tag="tmp")
                    tmp2 = ctmp2.tile([128, L - 1], F32, tag="tmp2")
                    # center tap + bias
                    nc.vector.tensor_scalar(
                        out=tmp, in0=xb, scalar1=w1, scalar2=bias,
                        op0=mybir.AluOpType.mult, op1=mybir.AluOpType.add)
                    # right tap: y[l] += w2 * x[l+1]  (l in 0..L-2)
                    nc.vector.tensor_scalar(
                        out=tmp2, in0=xb[:, 1:L], scalar1=w2, scalar2=None,
                        op0=mybir.AluOpType.mult)
                    nc.vector.tensor_add(tmp[:, 0:L - 1], tmp[:, 0:L - 1], tmp2)
                    # left tap: y[l] += w0 * x[l-1]  (l in 1..L-1)
                    nc.vector.tensor_scalar(
                        out=tmp2, in0=xb[:, 0:L - 1], scalar1=w0, scalar2=None,
                        op0=mybir.AluOpType.mult)
                    nc.vector.tensor_add(tmp[:, 1:L], tmp[:, 1:L], tmp2)
                    nc.scalar.copy(xb, tmp)

                for lt in range(NLT):
                    xt = xpool.tile([128, NDC, LT], F32R)
                    nc.sync.dma_start(
                        out=xt,
                        in_=xT.rearrange("(dc p) l -> p dc l", p=128)[
                            :, :, lt * LT:(lt + 1) * LT],
                    )
                    for pi in range(3):
                        ps = projps.tile([128, LT], F32)
                        for dc in range(NDC):
                            nc.tensor.matmul(
                                ps, wsb[pi][:, dc, :], xt[:, dc, :],
                                start=(dc == 0), stop=(dc == NDC - 1),
                            )
                        nc.scalar.copy(qkvT[pi][:, lt * LT:(lt + 1) * LT], ps)
                    if lt == 1:
                        # gate the attn zero-fill DMAs behind early projection
                        # output so they don't steal HBM bandwidth from the
                        # xT loads that feed the PE
                        nc.vector.tensor_scalar(
                            out=zero_sb, in0=qkvT[0][:, 0:1024],
                            scalar1=0.0, scalar2=None,
                            op0=mybir.AluOpType.mult)
                    if lt == NLT // 2 - 1:
                        # batch 0 fully projected: conv it now (v first — the
                        # attention phase consumes v earliest via transposes)
                        for pi in (2, 0, 1):
                            conv_one(pi, 0)
                for pi in (2, 0, 1):
                    conv_one(pi, 1)

            qT, kT, vT = qkvT

            # ---------------- attention ----------------
            with tc.tile_pool(name="v1p", bufs=2) as v1p, \
                 tc.tile_pool(name="expp", bufs=4) as expp, \
                 tc.tile_pool(name="sps", bufs=2, space="PSUM") as simps, \
                 tc.tile_pool(name="cps", bufs=2, space="PSUM") as ctxps, \
                 tc.tile_pool(name="ops", bufs=2, space="PSUM") as opps, \
                 tc.tile_pool(name="recp", bufs=2) as recp, \
                 tc.tile_pool(name="ctxtp", bufs=2) as ctxtp, \
                 tc.tile_pool(name="osbp", bufs=2) as osbp:
                for b in range(B):
                    # zero-fill of fully-masked attn regions (k > q), this batch
                    for h in range(HPC):
                        att_b = attnT[b, h].rearrange("(kt p) q -> p kt q", p=128)
                        for kt in range(4, NKT):
                            w = QC * (kt // 4)  # fully-masked columns 0..w
                            c0 = 0
                            while c0 < w:
                                c1 = min(c0 + 1024, w)
                                nc.gpsimd.dma_start(
                                    out=att_b[:, kt, c0:c1],
                                    in_=zero_sb[:, 0:c1 - c0])
                                c0 = c1

                    # natural-layout v tiles (+ ones column) for both heads
                    v1 = v1p.tile([128, HPC, NKT, HD + 1], F32R)
                    oc = ones_col[:, :]
                    oc_b = bass.AP(tensor=oc.tensor, offset=oc.offset,
                                   ap=[oc.ap[0], [0, HPC * NKT]])
                    v1_col = bass.AP(tensor=v1.tensor, offset=v1.offset + HD,
                                     ap=[v1.ap[0], [HD + 1, HPC * NKT]])
                    nc.sync.dma_start(out=v1_col, in_=oc_b)
                    for h in range(HPC):
                        for kt in range(NKT):
                            tp = simps.tile([128, HD], F32R, tag="sps")
                            nc.tensor.transpose(
                                tp,
                                vT[h * HD:(h + 1) * HD,
                                   b * L + kt * KT: b * L + (kt + 1) * KT],
                                ident_sb[h * HD:(h + 1) * HD, :])
                            nc.vector.tensor_copy(v1[:, h, kt, 0:HD], tp)

                    for a in range(NQC):
                        nkt = 4 * a + 4      # causal: k-tiles 0..nkt-1 computed
                        qsl = slice(b * L + a * QC, b * L + (a + 1) * QC)
                        ctx_t = ctxtp.tile([HD, HPC, QC], F32R)
                        # stage 1: sim + exp for BOTH heads (ACT-rate limited),
                        # then stage 2: both ctxt chains back-to-back — a dense
                        # PE block that keeps the HAM clock warm and overlaps
                        # head-0's recip/normalize tail with head-1's matmuls
                        ebufs = []
                        for h in range(HPC):
                            hsl = slice(h * HD, (h + 1) * HD)
                            # split the exp staging into <=8-k-tile subtiles:
                            # same SBUF footprint as one big buffer but pool
                            # slots recycle per-subtile, deepening the pipeline
                            sub = [expp.tile([128, min(8, nkt - 8 * i), QC],
                                             F32R, tag="ebuf", name=f"eb{i}")
                                   for i in range((nkt + 7) // 8)]
                            ebufs.append(sub)
                            for kp in range(0, nkt, 2):
                                sps = simps.tile([128, 2, QC], F32, tag="sps")
                                for j in range(2):
                                    kt = kp + j
                                    ksl = slice(b * L + kt * KT,
                                                b * L + (kt + 1) * KT)
                                    diag = kt >= nkt - 4
                                    nc.tensor.matmul(
                                        sps[:, j, :],
                                        kT[hsl, ksl], qT[hsl, qsl],
                                        start=True, stop=not diag)
                                    if diag:
                                        # sim += -1e30 on masked entries
                                        nc.tensor.matmul(
                                            sps[:, j, :], id128_sb,
                                            masks_sb[:, kt - (nkt - 4), :],
                                            start=False, stop=True)
                                nc.scalar.activation(
                                    sub[kp // 8][:, kp % 8:kp % 8 + 2, :], sps,
                                    mybir.ActivationFunctionType.Exp)
                        for h in range(HPC):
                            sub = ebufs[h]
                            # ctxt^T (rows 0..63) + softmax denominator (row 64)
                            cps = ctxps.tile([128, QC], F32)
                            for kt in range(nkt):
                                nc.tensor.matmul(
                                    cps[0:HD + 1, :],
                                    v1[:, h, kt, :],
                                    sub[kt // 8][:, kt % 8, :],
                                    start=(kt == 0), stop=(kt == nkt - 1))
                            # NB: reciprocal_approx_fast mis-reads nonzero base
                            # partitions on HW — copy the sum row to base 0 first.
                            srow = recp.tile([1, QC], F32, tag="srow")
                            nc.vector.tensor_copy(srow, cps[HD:HD + 1, :])
                            rec = recp.tile([1, QC], F32, tag="rec")
                            nc.vector.reciprocal_approx_fast(rec, srow)
                            # broadcast 1/sum across partitions via PE outer product
                            bc = opps.tile([128, QC], F32, tag="ops")
                            nc.tensor.matmul(bc, ones_sb, rec,
                                             start=True, stop=True)
                            bcs = recp.tile([128, QC], F32, tag="bcs")
                            nc.vector.tensor_copy(bcs, bc)
                            att_d = attnT[b, h].rearrange(
                                "(kt p) q -> p kt q", p=128)
                            for c0 in range(0, nkt, 4):
                                eb = sub[c0 // 8][:, c0 % 8:c0 % 8 + 4, :]
                                bcs_b = bass.AP(
                                    tensor=bcs.tensor, offset=bcs.offset,
                                    ap=[bcs.ap[0], [0, 4], bcs.ap[1]])
                                nc.vector.tensor_mul(eb, eb, bcs_b)
                                nc.sync.dma_start(
                                    out=att_d[:, c0:c0 + 4,
                                              a * QC:(a + 1) * QC],
                                    in_=eb.bitcast(F32))
                            # normalized ctxt^T for this head / q-chunk
                            nc.vector.tensor_mul(
                                ctx_t[:, h, :], cps[0:HD, :], bcs[0:HD, :])
                        # wo partial for this (b, q-chunk): contract local d
                        osb = None
                        for ec in range(NDC):
                            ops = opps.tile([128, QC], F32, tag="ops")
                            for h in range(HPC):
                                nc.tensor.matmul(
                                    ops,
                                    wo_sb[h][:, ec * 128:(ec + 1) * 128],
                                    ctx_t[:, h, :],
                                    start=(h == 0), stop=(h == HPC - 1))
                            if ec % 4 == 0:
                                osb = osbp.tile([128, 4, QC], F32, tag="osb")
                            nc.scalar.copy(osb[:, ec % 4, :], ops)
                            if ec % 4 == 3:
                                nc.sync.dma_start(
                                    out=outT.rearrange(
                                        "(ec p) l -> p ec l", p=128)[
                                            :, ec - 3:ec + 1,
                                            b * L + a * QC: b * L + (a + 1) * QC],
                                    in_=osb)
    nc.compile()
    return nc


_NC_CACHE = None


def _get_program():
    global _NC_CACHE
    if _NC_CACHE is None:
        _NC_CACHE = build_program()
    return _NC_CACHE


def make_in_maps(x, wq, wk, wv, wo, cq_w, cq_b, ck_w, ck_b, cv_w, cv_b):
    x2 = np.ascontiguousarray(x, dtype=np.float32).reshape(BL, D)
    xT = np.ascontiguousarray(x2.T)
    masks = np.zeros((4, 128, QC), dtype=np.float32)
    for d in range(4):
        p = np.arange(128)[:, None]
        j = np.arange(QC)[None, :]
        masks[d] = np.where(j >= p + 128 * d, 0.0, -1e30).astype(np.float32)
    ident = np.ascontiguousarray(np.tile(np.eye(HD, dtype=np.float32), (2, 1)))
    ones_r = np.ones((1, 128), dtype=np.float32)
    ones_col = np.ones((128, 1), dtype=np.float32)

    wk_s = (np.asarray(wk, dtype=np.float32) * SCALE)
    in_maps = []
    for c in range(NCORES):
        sl = slice(c * DL, (c + 1) * DL)
        woT = np.ascontiguousarray(
            np.stack([np.asarray(wo, dtype=np.float32)[:, c * DL + h * HD:
                                                       c * DL + (h + 1) * HD].T
                      for h in range(HPC)]))
        in_maps.append({
            "xT": xT,
            "wqT": np.ascontiguousarray(np.asarray(wq, dtype=np.float32)[sl].T),
            "wkT": np.ascontiguousarray(wk_s[sl].T),
            "wvT": np.ascontiguousarray(np.asarray(wv, dtype=np.float32)[sl].T),
            "cw": np.ascontiguousarray(np.stack([
                np.asarray(cq_w, dtype=np.float32)[sl, 0, :],
                np.asarray(ck_w, dtype=np.float32)[sl, 0, :],
                np.asarray(cv_w, dtype=np.float32)[sl, 0, :]])),
            "cb": np.ascontiguousarray(np.stack([
                np.asarray(cq_b, dtype=np.float32)[sl, None],
                np.asarray(ck_b, dtype=np.float32)[sl, None],
                np.asarray(cv_b, dtype=np.float32)[sl, None]])),
            "woT": woT,
            "masks": masks,
            "ident": ident,
            "ident128": np.eye(128, dtype=np.float32),
            "ones_r": ones_r,
            "ones_col": ones_col,
        })
    return in_maps


def gather(results):
    out_acc = np.zeros((D, BL), dtype=np.float64)
    attn = np.empty((B, H, L, L), dtype=np.float32)
    for c, res in enumerate(results):
        out_acc += res["outT"]
        attn[:, HPC * c:HPC * (c + 1)] = res["attnT"].transpose(0, 1, 3, 2)
    out = out_acc.astype(np.float32).T.reshape(B, L, D)
    return out, attn


def run(inputs, **spmd_kwargs):
    nc = _get_program()
    in_maps = make_in_maps(**inputs)
    br = run_bass_kernel_spmd(nc, in_maps, list(range(NCORES)), **spmd_kwargs)
    out, attn = gather(br.results)
    return (out, attn), br


def kernel(**inputs):
    (out, attn), _ = run(inputs)
    return out, attn


# revision 18
# speedup vs baseline: 1.1798x; 1.1393x over previous
"""Trainium2 Bass kernel for conv-augmented causal multi-head attention.

Module: q/k/v = Linear(x) (k scaled), depthwise conv(K=3) along L, causal MHA,
output projection. Returns (out [B,L,D], attn [B,H,L,L]).

Sharding: tensor-parallel over the 16 heads -> 2 heads per core (8 cores).
Each core computes its q/k/v slices [128, B*L] (transposed layout), runs
attention for its two heads with sim held TRANSPOSED ([k, q]) so that
  - attn @ v is a plain matmul with v in natural [k, d] layout (built via a
    PE transpose), and
  - the softmax denominator falls out of the same matmul via a ones column.
The wo partial products are summed on the host; attn slices are returned
transposed and swapped back on the host.

All matmuls run as float32r (full fp32 storage, relaxed-precision PE mode,
4x faster than strict fp32).
"""

import numpy as np

import concourse.bass as bass
import concourse.bacc as bacc_mod
import concourse.mybir as mybir
from concourse.tile import TileContext
from concourse.bass_utils import run_bass_kernel_spmd

B, L, D, H = 2, 2048, 1024, 16
HD = D // H                     # 64
NCORES = 8
HPC = H // NCORES               # 2 heads per core
DL = HPC * HD                   # 128 local channels per core
SCALE = HD ** -0.5
BL = B * L                      # 4096
F32 = mybir.dt.float32
F32R = mybir.dt.float32r

LT = 512                        # projection l-tile (moving dim)
NLT = BL // LT                  # 8
QC = 512                        # attention q chunk
NQC = L // QC                   # 4 per batch
KT = 128                        # attention k tile (partition dim)
NKT = L // KT                   # 16 per batch
NDC = D // 128                  # 8 contraction chunks


def _r(ap):
    return ap.bitcast(F32R)


def build_program():
    nc = bacc_mod.Bacc(None, target_bir_lowering=False, debug=False)

    xT = nc.declare_dram_parameter("xT", [D, BL], F32R, isOutput=False)
    wqT = nc.declare_dram_parameter("wqT", [D, DL], F32R, isOutput=False)
    wkT = nc.declare_dram_parameter("wkT", [D, DL], F32R, isOutput=False)
    wvT = nc.declare_dram_parameter("wvT", [D, DL], F32R, isOutput=False)
    cw = nc.declare_dram_parameter("cw", [3, DL, 3], F32, isOutput=False)
    cb = nc.declare_dram_parameter("cb", [3, DL, 1], F32, isOutput=False)
    woT = nc.declare_dram_parameter("woT", [HPC, HD, D], F32R, isOutput=False)
    masks = nc.declare_dram_parameter("masks", [4, 128, QC], F32R, isOutput=False)
    ident128 = nc.declare_dram_parameter("ident128", [128, 128], F32R, isOutput=False)
    ident = nc.declare_dram_parameter("ident", [128, HD], F32R, isOutput=False)
    ones_r = nc.declare_dram_parameter("ones_r", [1, 128], F32, isOutput=False)
    ones_col = nc.declare_dram_parameter("ones_col", [128, 1], F32R, isOutput=False)

    attnT = nc.declare_dram_parameter("attnT", [B, HPC, L, L], F32, isOutput=True)
    outT = nc.declare_dram_parameter("outT", [D, BL], F32, isOutput=True)

    with TileContext(nc) as tc:
        with tc.tile_pool(name="consts", bufs=1) as consts:
            # constants
            wsb = []
            for i, w in enumerate((wqT, wkT, wvT)):
                t = consts.tile([128, NDC, DL], F32R, name=f"w_{i}")
                nc.sync.dma_start(out=t, in_=w.rearrange("(dc p) e -> p dc e", p=128))
                wsb.append(t)
            cw_sb = consts.tile([DL, 3, 3], F32)
            nc.sync.dma_start(out=cw_sb, in_=cw.rearrange("m p k -> p m k"))
            cb_sb = consts.tile([DL, 3, 1], F32)
            nc.sync.dma_start(out=cb_sb, in_=cb.rearrange("m p k -> p m k"))
            wo_sb = []
            for h in range(HPC):
                t = consts.tile([HD, D], F32R, name=f"wo_{h}")
                nc.sync.dma_start(out=t, in_=woT[h])
                wo_sb.append(t)
            masks_sb = consts.tile([128, 4, QC], F32R)
            nc.sync.dma_start(out=masks_sb, in_=masks.rearrange("m p q -> p m q"))
            ident_sb = consts.tile([128, HD], F32R)
            nc.sync.dma_start(out=ident_sb, in_=ident[:, :])
            id128_sb = consts.tile([128, 128], F32R)
            nc.sync.dma_start(out=id128_sb, in_=ident128[:, :])
            ones_sb = consts.tile([1, 128], F32)
            nc.sync.dma_start(out=ones_sb, in_=ones_r[:, :])
            zero_sb = consts.tile([128, 1024], F32)

            # persistent projection outputs [128ch, B*L]
            qkvT = []
            for i in range(3):
                t = consts.tile([128, BL], F32R, name=f"qkvT_{i}")
                qkvT.append(t)

            # ---------------- projections ----------------
            with tc.tile_pool(name="proj_x", bufs=2) as xpool, \
                 tc.tile_pool(name="proj_ps", bufs=2, space="PSUM") as projps, \
                 tc.tile_pool(name="conv_tmp", bufs=2) as ctmp, \
                 tc.tile_pool(name="conv_tmp2", bufs=2) as ctmp2:

                def conv_one(pi, b):
                    w0 = cw_sb[:, pi, 0:1]
                    w1 = cw_sb[:, pi, 1:2]
                    w2 = cw_sb[:, pi, 2:3]
                    bias = cb_sb[:, pi, :]
                    xb = qkvT[pi][:, b * L:(b + 1) * L]
                    tmp = ctmp.tile([128, L], F32, tag="tmp")
                    tmp2 = ctmp2.tile([128, L - 1], F32, tag="tmp2")
                    # center tap + bias
                    nc.vector.tensor_scalar(
                        out=tmp, in0=xb, scalar1=w1, scalar2=bias,
                        op0=mybir.AluOpType.mult, op1=mybir.AluOpType.add)
                    # right tap: y[l] += w2 * x[l+1]  (l in 0..L-2)
                    nc.vector.tensor_scalar(
                        out=tmp2, in0=xb[:, 1:L], scalar1=w2, scalar2=None,
                        op0=mybir.AluOpType.mult)
                    nc.vector.tensor_add(tmp[:, 0:L - 1], tmp[:, 0:L - 1], tmp2)
                    # left tap: y[l] += w0 * x[l-1]  (l in 1..L-1)
                    nc.vector.tensor_scalar(
                        out=tmp2, in0=xb[:, 0:L - 1], scalar1=w0, scalar2=None,
                        op0=mybir.AluOpType.mult)
                    nc.vector.tensor_add(tmp[:, 1:L], tmp[:, 1:L], tmp2)
                    nc.scalar.copy(xb, tmp)

                for lt in range(NLT):
                    xt = xpool.tile([128, NDC, LT], F32R)
                    nc.sync.dma_start(
                        out=xt,
                        in_=xT.rearrange("(dc p) l -> p dc l", p=128)[
                            :, :, lt * LT:(lt + 1) * LT],
                    )
                    for pi in range(3):
                        ps = projps.tile([128, LT], F32)
                        for dc in range(NDC):
                            nc.tensor.matmul(
                                ps, wsb[pi][:, dc, :], xt[:, dc, :],
                                start=(dc == 0), stop=(dc == NDC - 1),
                            )
                        nc.scalar.copy(qkvT[pi][:, lt * LT:(lt + 1) * LT], ps)
                    if lt == NLT - 1:
                        # gate the attn zero-fill DMAs behind the LAST xT load
                        # so the projection phase keeps full HBM read bandwidth
                        # (the attention window absorbs the zero writes)
                        nc.vector.tensor_scalar(
                            out=zero_sb, in0=qkvT[0][:, 0:1024],
                            scalar1=0.0, scalar2=None,
                            op0=mybir.AluOpType.mult)
                    if lt == NLT // 2 - 1:
                        # batch 0 fully projected: conv it now (v first — the
                        # attention phase consumes v earliest via transposes)
                        for pi in (2, 0, 1):
                            conv_one(pi, 0)
                for pi in (2, 0, 1):
                    conv_one(pi, 1)

            qT, kT, vT = qkvT

            # ---------------- attention ----------------
            with tc.tile_pool(name="v1p", bufs=2) as v1p, \
                 tc.tile_pool(name="expp", bufs=4) as expp, \
                 tc.tile_pool(name="sps", bufs=2, space="PSUM") as simps, \
                 tc.tile_pool(name="cps", bufs=2, space="PSUM") as ctxps, \
                 tc.tile_pool(name="ops", bufs=2, space="PSUM") as opps, \
                 tc.tile_pool(name="recp", bufs=2) as recp, \
                 tc.tile_pool(name="ctxtp", bufs=2) as ctxtp, \
                 tc.tile_pool(name="osbp", bufs=2) as osbp:
                for b in range(B):
                    # zero-fill of fully-masked attn regions (k > q), this batch
                    for h in range(HPC):
                        att_b = attnT[b, h].rearrange("(kt p) q -> p kt q", p=128)
                        for kt in range(4, NKT):
                            w = QC * (kt // 4)  # fully-masked columns 0..w
                            c0 = 0
                            while c0 < w:
                                c1 = min(c0 + 1024, w)
                                nc.gpsimd.dma_start(
                                    out=att_b[:, kt, c0:c1],
                                    in_=zero_sb[:, 0:c1 - c0])
                                c0 = c1

                    # natural-layout v tiles (+ ones column) for both heads
                    v1 = v1p.tile([128, HPC, NKT, HD + 1], F32R)
                    oc = ones_col[:, :]
                    oc_b = bass.AP(tensor=oc.tensor, offset=oc.offset,
                                   ap=[oc.ap[0], [0, HPC * NKT]])
                    v1_col = bass.AP(tensor=v1.tensor, offset=v1.offset + HD,
                                     ap=[v1.ap[0], [HD + 1, HPC * NKT]])
                    nc.sync.dma_start(out=v1_col, in_=oc_b)
                    for h in range(HPC):
                        for kt in range(NKT):
                            tp = simps.tile([128, HD], F32R, tag="sps")
                            nc.tensor.transpose(
                                tp,
                                vT[h * HD:(h + 1) * HD,
                                   b * L + kt * KT: b * L + (kt + 1) * KT],
                                ident_sb[h * HD:(h + 1) * HD, :])
                            nc.vector.tensor_copy(v1[:, h, kt, 0:HD], tp)

                    for a in range(NQC):
                        nkt = 4 * a + 4      # causal: k-tiles 0..nkt-1 computed
                        qsl = slice(b * L + a * QC, b * L + (a + 1) * QC)
                        ctx_t = ctxtp.tile([HD, HPC, QC], F32R)
                        # stage 1: sim + exp for BOTH heads (ACT-rate limited),
                        # then stage 2: both ctxt chains back-to-back — a dense
                        # PE block that keeps the HAM clock warm and overlaps
                        # head-0's recip/normalize tail with head-1's matmuls
                        ebufs = []
                        for h in range(HPC):
                            hsl = slice(h * HD, (h + 1) * HD)
                            # split the exp staging into <=8-k-tile subtiles:
                            # same SBUF footprint as one big buffer but pool
                            # slots recycle per-subtile, deepening the pipeline
                            sub = [expp.tile([128, min(8, nkt - 8 * i), QC],
                                             F32R, tag="ebuf", name=f"eb{i}")
                                   for i in range((nkt + 7) // 8)]
                            ebufs.append(sub)
                            for kp in range(0, nkt, 2):
                                sps = simps.tile([128, 2, QC], F32, tag="sps")
                                for j in range(2):
                                    kt = kp + j
                                    ksl = slice(b * L + kt * KT,
                                                b * L + (kt + 1) * KT)
                                    diag = kt >= nkt - 4
                                    nc.tensor.matmul(
                                        sps[:, j, :],
                                        kT[hsl, ksl], qT[hsl, qsl],
                                        start=True, stop=not diag)
                                    if diag:
                                        # sim += -1e30 on masked entries
                                        nc.tensor.matmul(
                                            sps[:, j, :], id128_sb,
                                            masks_sb[:, kt - (nkt - 4), :],
                                            start=False, stop=True)
                                nc.scalar.activation(
                                    sub[kp // 8][:, kp % 8:kp % 8 + 2, :], sps,
                                    mybir.ActivationFunctionType.Exp)
                        for h in range(HPC):
                            sub = ebufs[h]
                            # ctxt^T (rows 0..63) + softmax denominator (row 64)
                            cps = ctxps.tile([128, QC], F32)
                            for kt in range(nkt):
                                nc.tensor.matmul(
                                    cps[0:HD + 1, :],
                                    v1[:, h, kt, :],
                                    sub[kt // 8][:, kt % 8, :],
                                    start=(kt == 0), stop=(kt == nkt - 1))
                            # NB: reciprocal_approx_fast mis-reads nonzero base
                            # partitions on HW — copy the sum row to base 0 first.
                            srow = recp.tile([1, QC], F32, tag="srow")
                            nc.vector.tensor_copy(srow, cps[HD:HD + 1, :])
                            rec = recp.tile([1, QC], F32, tag="rec")
                            nc.vector.reciprocal_approx_fast(rec, srow)
                            # broadcast 1/sum across partitions via PE outer product
                            bc = opps.tile([128, QC], F32, tag="ops")
                            nc.tensor.matmul(bc, ones_sb, rec,
                                             start=True, stop=True)
                            bcs = recp.tile([128, QC], F32, tag="bcs")
                            nc.vector.tensor_copy(bcs, bc)
                            att_d = attnT[b, h].rearrange(
                                "(kt p) q -> p kt q", p=128)
                            for c0 in range(0, nkt, 4):
                                eb = sub[c0 // 8][:, c0 % 8:c0 % 8 + 4, :]
                                bcs_b = bass.AP(
                                    tensor=bcs.tensor, offset=bcs.offset,
                                    ap=[bcs.ap[0], [0, 4], bcs.ap[1]])
                                nc.vector.tensor_mul(eb, eb, bcs_b)
                                nc.sync.dma_start(
                                    out=att_d[:, c0:c0 + 4,
                                              a * QC:(a + 1) * QC],
                                    in_=eb.bitcast(F32))
                            # normalized ctxt^T for this head / q-chunk
                            nc.vector.tensor_mul(
                                ctx_t[:, h, :], cps[0:HD, :], bcs[0:HD, :])
                        # wo partial for this (b, q-chunk): contract local d
                        osb = None
                        for ec in range(NDC):
                            ops = opps.tile([128, QC], F32, tag="ops")
                            for h in range(HPC):
                                nc.tensor.matmul(
                                    ops,
                                    wo_sb[h][:, ec * 128:(ec + 1) * 128],
                                    ctx_t[:, h, :],
                                    start=(h == 0), stop=(h == HPC - 1))
                            if ec % 4 == 0:
                                osb = osbp.tile([128, 4, QC], F32, tag="osb")
                            nc.vector.tensor_copy(osb[:, ec % 4, :], ops)
                            if ec % 4 == 3:
                                nc.sync.dma_start(
                                    out=outT.rearrange(
                                        "(ec p) l -> p ec l", p=128)[
                                            :, ec - 3:ec + 1,
                                            b * L + a * QC: b * L + (a + 1) * QC],
                                    in_=osb)
    nc.compile()
    return nc


_NC_CACHE = None


def _get_program():
    global _NC_CACHE
    if _NC_CACHE is None:
        _NC_CACHE = build_program()
    return _NC_CACHE


def make_in_maps(x, wq, wk, wv, wo, cq_w, cq_b, ck_w, ck_b, cv_w, cv_b):
    x2 = np.ascontiguousarray(x, dtype=np.float32).reshape(BL, D)
    xT = np.ascontiguousarray(x2.T)
    masks = np.zeros((4, 128, QC), dtype=np.float32)
    for d in range(4):
        p = np.arange(128)[:, None]
        j = np.arange(QC)[None, :]
        masks[d] = np.where(j >= p + 128 * d, 0.0, -1e30).astype(np.float32)
    ident = np.ascontiguousarray(np.tile(np.eye(HD, dtype=np.float32), (2, 1)))
    ones_r = np.ones((1, 128), dtype=np.float32)
    ones_col = np.ones((128, 1), dtype=np.float32)

    wk_s = (np.asarray(wk, dtype=np.float32) * SCALE)
    in_maps = []
    for c in range(NCORES):
        sl = slice(c * DL, (c + 1) * DL)
        woT = np.ascontiguousarray(
            np.stack([np.asarray(wo, dtype=np.float32)[:, c * DL + h * HD:
                                                       c * DL + (h + 1) * HD].T
                      for h in range(HPC)]))
        in_maps.append({
            "xT": xT,
            "wqT": np.ascontiguousarray(np.asarray(wq, dtype=np.float32)[sl].T),
            "wkT": np.ascontiguousarray(wk_s[sl].T),
            "wvT": np.ascontiguousarray(np.asarray(wv, dtype=np.float32)[sl].T),
            "cw": np.ascontiguousarray(np.stack([
                np.asarray(cq_w, dtype=np.float32)[sl, 0, :],
                np.asarray(ck_w, dtype=np.float32)[sl, 0, :],
                np.asarray(cv_w, dtype=np.float32)[sl, 0, :]])),
            "cb": np.ascontiguousarray(np.stack([
                np.asarray(cq_b, dtype=np.float32)[sl, None],
                np.asarray(ck_b, dtype=np.float32)[sl, None],
                np.asarray(cv_b, dtype=np.float32)[sl, None]])),
            "woT": woT,
            "masks": masks,
            "ident": ident,
            "ident128": np.eye(128, dtype=np.float32),
            "ones_r": ones_r,
            "ones_col": ones_col,
        })
    return in_maps


def gather(results):
    out_acc = np.zeros((D, BL), dtype=np.float64)
    attn = np.empty((B, H, L, L), dtype=np.float32)
    for c, res in enumerate(results):
        out_acc += res["outT"]
        attn[:, HPC * c:HPC * (c + 1)] = res["attnT"].transpose(0, 1, 3, 2)
    out = out_acc.astype(np.float32).T.reshape(B, L, D)
    return out, attn


def run(inputs, **spmd_kwargs):
    nc = _get_program()
    in_maps = make_in_maps(**inputs)
    br = run_bass_kernel_spmd(nc, in_maps, list(range(NCORES)), **spmd_kwargs)
    out, attn = gather(br.results)
    return (out, attn), br


def kernel(**inputs):
    (out, attn), _ = run(inputs)
    return out, attn
